# revision 31
# baseline (speedup 1.0000x reference)
"""Trainium2 Bass kernel for nn_MultiHeadAttention_5334349382389.

Sharding: 8 cores = 4 batches x 2 head-groups (4 heads each).
Core c handles batch b = c // 2, head-group g = c % 2 (heads 4g..4g+3).

Per-core math (matmuls in fp16 by default, fp32 PSUM accumulate):
  qhT = (Wq_g/8) @ x_b^T + bq_g/8        [256, 1024]   (score scale folded into Wq)
  khT = Wk_g @ x_b^T + bk_g              [256, 1024]
  vh  = x_b @ Wv_g^T                     [1024, 256]   (bv folded into host-side bias)
  per head h: scoresT[k,q] via a K=128 matmul against zero-padded khp (full
      PE-array activity keeps the HAM clock at 2.4 GHz), plus I @ edgeT
      accumulated in PSUM (edgeT is zeros on non-edge cores; Wq/bq head-0
      slice zeroed on edge cores, so edge cores get scoresT == edgeT exactly)
  expT = exp(scoresT)                    (no max-subtraction; inputs are bounded)
  outT_raw[d,q] accum over k-tiles with lhsT = [vh | ones] -> row 64 = softmax denom
  OT = outT_raw[:64] * bcast(1/denom)
  partial = OT^T-contraction @ WoT_g     [1024, 512]

Host: transposes/slices/casts inputs per core, gathers
  out[b] = partial(b,0) + partial(b,1) + (bo + Wo @ bv)
(the bv term is exact because softmax rows sum to 1).
"""

import os
import sys

sys.path.insert(0, "/opt/trn_rl_repo")

import numpy as np

B, SEQ, DIN, DO = 4, 1024, 512, 512
NH_ALL, DK = 8, 64
NHC = 4            # heads per core
DH = NHC * DK      # 256 per-core projected dims
P = 128
CD = DIN // P      # 4 contraction chunks for projections
CH = DH // P       # 2 dh chunks
KT = SEQ // P      # 8 k-tiles
STR = 512          # q-stripe (matmul free dim)
NS = SEQ // STR    # 2 stripes
TVW = NHC * (DK + 1) + DK - 1  # 323: per-k-tile aux width (4x65 + 63 pad)
NPS = P + CD * DH + CD * SEQ             # eye | wq | xq
NPK = CD * DH + CD * SEQ                 # wk | xk
NPV = CD * DH + CD * SEQ + CH * DO        # wv | xv | wo
NPA = KT * TVW                            # vaux

COMPUTE = os.environ.get("KERNEL_COMPUTE_DT", "fp16")  # fp16 | bf16 | fp32r

_nc = None


def _np_dt():
    import ml_dtypes

    return {
        "fp16": np.float16,
        "bf16": ml_dtypes.bfloat16,
        "fp32r": np.float32,
    }[COMPUTE]


def _build():
    global _nc
    if _nc is not None:
        return _nc
    import concourse.bacc as bacc
    import concourse.bass as bass
    import concourse.mybir as mybir
    import concourse.tile as tile

    f32 = mybir.dt.float32
    f32r = mybir.dt.float32r
    cdt = {
        "fp16": mybir.dt.float16,
        "bf16": mybir.dt.bfloat16,
        "fp32r": f32r,
    }[COMPUTE]
    Exp = mybir.ActivationFunctionType.Exp

    nc = bacc.Bacc("TRN2", target_bir_lowering=False, debug=False)

    pk_s = nc.dram_tensor("pk_s", (P, NPS), cdt, kind="ExternalInput")
    pk_k = nc.dram_tensor("pk_k", (P, NPK), cdt, kind="ExternalInput")
    pk_v = nc.dram_tensor("pk_v", (P, NPV), cdt, kind="ExternalInput")
    pk_a = nc.dram_tensor("pk_a", (P, NPA), cdt, kind="ExternalInput")
    bqk = nc.dram_tensor("bqk", (2 * DH, 1), f32, kind="ExternalInput")
    edge = nc.dram_tensor("edge", (SEQ, SEQ), cdt, kind="ExternalInput")
    outp = nc.dram_tensor("outp", (SEQ, DO), cdt, kind="ExternalOutput")

    edge_r = edge.rearrange("(t p) n -> t p n", p=P)

    def sl(s):
        return slice(s * STR, (s + 1) * STR)

    with tile.TileContext(nc) as tc:
        with (
            tc.tile_pool(name="inp", bufs=1) as inp,
            tc.tile_pool(name="wts", bufs=1) as wts,
            tc.tile_pool(name="qkp", bufs=1) as qkp,
            tc.tile_pool(name="vhap", bufs=1) as vhap,
            tc.tile_pool(name="expp", bufs=12) as expp,
            tc.tile_pool(name="otp", bufs=1) as otp,
            tc.tile_pool(name="rrp", bufs=4) as rrp,
            tc.tile_pool(name="rbp", bufs=4) as rbp,
            tc.tile_pool(name="outsp", bufs=1) as outsp,
            tc.tile_pool(name="edgp", bufs=6) as edgp,
            tc.tile_pool(name="bigp", bufs=2, space=bass.MemorySpace.PSUM) as bigp,
            tc.tile_pool(name="povp", bufs=3, space=bass.MemorySpace.PSUM) as povp,
            tc.tile_pool(name="jnkp", bufs=1, space=bass.MemorySpace.PSUM) as jnkp,
        ):
            # ------- input loads: one big packed DMA per ring -------
            tps = inp.tile([P, NPS], cdt, tag="tps")
            tpk = inp.tile([P, NPK], cdt, tag="tpk")
            tpv = inp.tile([P, NPV], cdt, tag="tpv")
            tvha = vhap.tile([P, KT, TVW], cdt, tag="tvha")
            HS, HK, HV = NPS // 2, NPK // 2, NPV // 2
            nc.sync.dma_start(out=tps[:, 0:HS], in_=pk_s[:, 0:HS])
            nc.scalar.dma_start(out=tps[:, HS:NPS], in_=pk_s[:, HS:NPS])
            nc.sync.dma_start(out=tpk[:, 0:HK], in_=pk_k[:, 0:HK])
            nc.scalar.dma_start(out=tpk[:, HK:NPK], in_=pk_k[:, HK:NPK])
            nc.scalar.dma_start(out=tpv[:, 0:HV], in_=pk_v[:, 0:HV])
            nc.gpsimd.dma_start(out=tpv[:, HV:NPV], in_=pk_v[:, HV:NPV])
            nc.gpsimd.dma_start(
                out=tvha, in_=pk_a.rearrange("p (t w) -> p t w", w=TVW)
            )
            tb4 = wts.tile([P, 4, 1], f32, tag="tb4")
            nc.sync.dma_start(out=tb4, in_=bqk.rearrange("(c p) o -> p c o", p=P))

            teye = tps[:, 0:P]
            twq = tps[:, P : P + CD * DH].rearrange("p (c d) -> p c d", d=DH)
            txq = tps[:, P + CD * DH : P + CD * DH + CD * SEQ].rearrange(
                "p (c n) -> p c n", n=SEQ
            )
            twk = tpk[:, 0 : CD * DH].rearrange("p (c d) -> p c d", d=DH)
            txk = tpk[:, CD * DH : CD * DH + CD * SEQ].rearrange(
                "p (c n) -> p c n", n=SEQ
            )
            twv = tpv[:, 0 : CD * DH].rearrange("p (c d) -> p c d", d=DH)
            txv = tpv[:, CD * DH : CD * DH + CD * SEQ].rearrange(
                "p (c n) -> p c n", n=SEQ
            )
            two = tpv[:, CD * DH + CD * SEQ : NPV].rearrange("p (c d) -> p c d", d=DO)

            # PE warmup/filler: junk matmuls on the early-arriving identity
            # tile keep the HAM activity window busy (2.4 GHz clock) while DMAs
            # land and across phase boundaries. They use a dedicated PSUM bank
            # that is never read.
            jnk = jnkp.tile([P, STR], f32, tag="jnk")

            def junk(n):
                for _ in range(n):
                    nc.tensor.matmul(
                        jnk[:, 0:P], lhsT=teye[:], rhs=teye[:], start=True, stop=True
                    )

            junk(34)

            # ---------------- projections ----------------
            tqh = qkp.tile([P, CH, SEQ], cdt, tag="tqh")
            khp = qkp.tile([P, NHC, SEQ], cdt, tag="khp")
            nc.gpsimd.memset(khp[0:DK, 1 :: 2, :], 0.0)
            nc.gpsimd.memset(khp[DK:P, 0 :: 2, :], 0.0)
            # zero the unused partition-halves of khp (even heads: parts 64-127,
            # odd heads: parts 0-63) so K=128 score matmuls see zero weights there
            def proj_q(ch):
                pt = bigp.tile([P, SEQ], f32, tag="big")
                for cd in range(CD):
                    for s in range(NS):
                        nc.tensor.matmul(
                            pt[:, sl(s)],
                            lhsT=twq[:, cd, ch * P : (ch + 1) * P],
                            rhs=txq[:, cd, sl(s)],
                            start=(cd == 0),
                            stop=(cd == CD - 1),
                        )
                    if ch == 0:
                        junk(4)
                nc.vector.tensor_scalar_add(
                    out=tqh[:, ch, :], in0=pt[:], scalar1=tb4[:, ch, :]
                )

            def proj_k(ch):
                pt = bigp.tile([P, SEQ], f32, tag="big")
                for cd in range(CD):
                    for s in range(NS):
                        nc.tensor.matmul(
                            pt[:, sl(s)],
                            lhsT=twk[:, cd, ch * P : (ch + 1) * P],
                            rhs=txk[:, cd, sl(s)],
                            start=(cd == 0),
                            stop=(cd == CD - 1),
                        )
                    if ch == 0:
                        junk(4)
                nc.vector.tensor_scalar_add(
                    out=khp[0:DK, 2 * ch, :],
                    in0=pt[0:DK, :],
                    scalar1=tb4[0:DK, 2 + ch, :],
                )
                nc.vector.tensor_scalar_add(
                    out=khp[DK:P, 2 * ch + 1, :],
                    in0=pt[DK:P, :],
                    scalar1=tb4[DK:P, 2 + ch, :],
                )

            proj_q(0)
            proj_k(0)

            # v: [s, dh] tiles written into vh_aug (65-wide per head, col 64 = 1.0)
            for st in range(KT):
                pt = bigp.tile([P, SEQ], f32, tag="big")
                for cd in range(CD):
                    nc.tensor.matmul(
                        pt[:, 0:DH],
                        lhsT=txv[:, cd, st * P : (st + 1) * P],
                        rhs=twv[:, cd, :],
                        start=(cd == 0),
                        stop=(cd == CD - 1),
                    )
                nc.vector.tensor_copy(
                    out=tvha[:, st, 0 : NHC * (DK + 1)].rearrange(
                        "p (h w) -> p h w", w=DK + 1
                    )[:, :, 0:DK],
                    in_=pt[:, 0:DH].rearrange("p (h d) -> p h d", h=NHC),
                )

            # ---------------- attention per head ----------------
            tot = otp.tile([P, CH, SEQ], cdt, tag="tot")

            def head_body(h):
                ch, off = h // 2, (h % 2) * DK
                pv0 = povp.tile([P, STR], f32, tag="pov")
                pv1 = povp.tile([P, STR], f32, tag="pov")
                pvs = (pv0, pv1)
                for kt in range(KT):
                    stt = bigp.tile([P, SEQ], f32, tag="big")
                    ed = None
                    if h == 0:
                        ed = edgp.tile([P, SEQ], cdt, tag="edg")
                        eng = (nc.sync, nc.sync, nc.scalar, nc.scalar)[kt % 4]
                        eng.dma_start(out=ed, in_=edge_r[kt])
                    for s in range(NS):
                        nc.tensor.matmul(
                            stt[:, sl(s)],
                            lhsT=khp[:, h, kt * P : (kt + 1) * P],
                            rhs=tqh[:, ch, sl(s)],
                            start=True,
                            stop=(h != 0),
                        )
                        if h == 0:
                            nc.tensor.matmul(
                                stt[:, sl(s)],
                                lhsT=teye[:],
                                rhs=ed[:, sl(s)],
                                start=False,
                                stop=True,
                            )
                    te = expp.tile([P, SEQ], cdt, tag="expT")
                    nc.scalar.activation(out=te, in_=stt[:], func=Exp)
                    for s in range(NS):
                        nc.tensor.matmul(
                            pvs[s][:, :],
                            lhsT=tvha[:, kt, h * (DK + 1) : h * (DK + 1) + P],
                            rhs=te[:, sl(s)],
                            start=(kt == 0),
                            stop=(kt == KT - 1),
                        )
                for s in range(NS):
                    rr = rrp.tile([1, STR], f32, tag="rr")
                    rs = rrp.tile([1, STR], f32, tag="rs")
                    nc.vector.tensor_copy(out=rs[:], in_=pvs[s][DK : DK + 1, :])
                    nc.vector.reciprocal_approx_fast(out=rr[:], in_=rs[:])
                    rb = rbp.tile([DK, STR], f32, tag="rb")
                    nc.gpsimd.partition_broadcast(rb[:], rr[:])
                    nc.vector.tensor_mul(
                        tot[off : off + DK, ch, sl(s)], pvs[s][0:DK, :], rb[:]
                    )

            head_body(0)
            proj_q(1)
            proj_k(1)
            head_body(1)
            junk(24)
            head_body(2)
            junk(24)
            head_body(3)

            # ---------------- output projection ----------------
            oall = outsp.tile([P, KT, DO], cdt, tag="oall")
            for m in range(KT):
                po = bigp.tile([P, SEQ], f32, tag="big")
                for i, ch in enumerate((0, 1)):
                    nc.tensor.matmul(
                        po[:, 0:DO],
                        lhsT=tot[:, ch, m * P : (m + 1) * P],
                        rhs=two[:, ch, :],
                        start=(i == 0),
                        stop=(i == CH - 1),
                    )
                nc.vector.tensor_copy(out=oall[:, m, :], in_=po[:, 0:DO])
                if m == KT // 2 - 1:
                    nc.sync.dma_start(
                        out=outp.rearrange("(t p) n -> p t n", p=P)[:, 0 : KT // 2, :],
                        in_=oall[:, 0 : KT // 2, :],
                    )
            nc.scalar.dma_start(
                out=outp.rearrange("(t p) n -> p t n", p=P)[:, KT // 2 : KT, :],
                in_=oall[:, KT // 2 : KT, :],
            )

    nc.compile()
    _nc = nc
    return nc


def _in_maps(q, k, v, edge_matrix, Wq, bq, Wk, bk, Wv, Wo):
    dt = _np_dt()
    zeros_edge = np.zeros((SEQ, SEQ), dt)
    edge_t = np.ascontiguousarray(edge_matrix.T).astype(dt)
    ident = np.eye(P, dtype=dt)
    vaux_in = np.zeros((P, NPA), dt)
    for h in range(NHC):
        vaux_in[:, h * (DK + 1) + DK :: TVW] = 1.0

    def re_cp(m):
        # [C*P, D] -> [P, C*D] (partition-major packing of "(c p) d -> p c d")
        cp, d = m.shape
        return np.ascontiguousarray(
            m.reshape(cp // P, P, d).transpose(1, 0, 2).reshape(P, -1)
        )

    xt = {}
    for b in range(B):
        xt[b] = (
            re_cp(np.ascontiguousarray(q[b].T).astype(dt)),
            re_cp(np.ascontiguousarray(k[b].T).astype(dt)),
            re_cp(np.ascontiguousarray(v[b].T).astype(dt)),
        )
    maps = []
    for c in range(8):
        b, g = c // 2, c % 2
        is_edge = g == 0 and b < 2
        rows = slice(g * DH, (g + 1) * DH)
        wq_c = np.ascontiguousarray(Wq[rows].T) * np.float32(1.0 / 8.0)
        bq_c = (bq[rows] * np.float32(1.0 / 8.0)).copy()
        if is_edge:
            wq_c[:, 0:DK] = 0.0
            bq_c[0:DK] = 0.0
        pks = np.concatenate([ident, re_cp(wq_c.astype(dt)), xt[b][0]], axis=1)
        pkk = np.concatenate(
            [re_cp(np.ascontiguousarray(Wk[rows].T).astype(dt)), xt[b][1]], axis=1
        )
        pkv = np.concatenate(
            [
                re_cp(np.ascontiguousarray(Wv[rows].T).astype(dt)),
                xt[b][2],
                re_cp(np.ascontiguousarray(Wo[:, rows].T).astype(dt)),
            ],
            axis=1,
        )
        assert pks.shape == (P, NPS) and pkk.shape == (P, NPK)
        assert pkv.shape == (P, NPV)
        maps.append(
            {
                "pk_s": np.ascontiguousarray(pks),
                "pk_k": np.ascontiguousarray(pkk),
                "pk_v": np.ascontiguousarray(pkv),
                "pk_a": vaux_in,
                "bqk": np.concatenate([bq_c, bk[rows]]).reshape(2 * DH, 1),
                "edge": edge_t if is_edge else zeros_edge,
            }
        )
    return maps


def _ensure_ntff_hook():
    """Register the axon NTFF profile hook if the image's antenv lacks it."""
    import contextlib
    import ctypes
    import types

    try:
        from antenv.axon_hooks import get_axon_ntff_profile_hook  # noqa: F401
        return
    except ImportError:
        pass

    so_path = "/opt/axon/libaxon_pjrt.so"
    try:
        lib = ctypes.CDLL(so_path)
    except OSError:
        return
    if not hasattr(lib, "axon_start_nrt_profile"):
        return
    lib.axon_start_nrt_profile.argtypes = [
        ctypes.POINTER(ctypes.c_int64),
        ctypes.c_size_t,
    ]
    lib.axon_start_nrt_profile.restype = ctypes.c_int64
    lib.axon_stop_nrt_profile.argtypes = [ctypes.c_char_p]
    lib.axon_stop_nrt_profile.restype = ctypes.c_int64

    @contextlib.contextmanager
    def _hook(output_dir, device_ids):
        import jax

        jax.devices()
        if device_ids:
            ids = (ctypes.c_int64 * len(device_ids))(*device_ids)
            rc = lib.axon_start_nrt_profile(ids, len(device_ids))
        else:
            rc = lib.axon_start_nrt_profile(None, 0)
        if rc != 0:
            raise RuntimeError(f"axon_start_nrt_profile rc={rc}")
        try:
            yield
        finally:
            n = lib.axon_stop_nrt_profile(str(output_dir).encode())
            if n < 0:
                raise RuntimeError(f"axon_stop_nrt_profile rc={n}")

    _state = {"hook": _hook}
    mod = types.ModuleType("antenv.axon_hooks")
    mod.get_axon_ntff_profile_hook = lambda: _state["hook"]
    mod.set_axon_ntff_profile_hook = lambda h: _state.__setitem__("hook", h)
    import antenv

    antenv.axon_hooks = mod
    sys.modules["antenv.axon_hooks"] = mod


def kernel(q, k, v, edge_matrix, Wq, bq, Wk, bk, Wv, bv, Wo, bo, _trace=False):
    from concourse.bass_utils import run_bass_kernel_spmd

    if _trace:
        _ensure_ntff_hook()

    q, k, v = (np.asarray(t, np.float32) for t in (q, k, v))
    edge_matrix = np.asarray(edge_matrix, np.float32)
    Wq, bq, Wk, bk, Wv, bv, Wo, bo = (
        np.asarray(t, np.float32) for t in (Wq, bq, Wk, bk, Wv, bv, Wo, bo)
    )

    nc = _build()
    maps = _in_maps(q, k, v, edge_matrix, Wq, bq, Wk, bk, Wv, Wo)
    res = run_bass_kernel_spmd(nc, maps, core_ids=list(range(8)), trace=_trace)

    bo_eff = bo + Wo @ bv
    out = np.empty((B, SEQ, DO), np.float32)
    for b in range(B):
        out[b] = res.results[2 * b]["outp"] + res.results[2 * b + 1]["outp"] + bo_eff
    if _trace:
        return out, res
    return out
